# revision 36
# baseline (speedup 1.0000x reference)
"""MCR2 loss kernel for 8 Trainium2 NeuronCores.

Host-side counting sort by class label removes all masking work from the
device: each core receives its share of every class's rows, zero-padded
to 512-row (quad) alignment and pre-packed in fp8 e4m3 (the logdet
difference loss_R - loss_Rc cancels the correlated quantization error;
measured rel err 3.1e-3, on par with bf16, for half the HBM traffic).
A quad is 4 sample tiles of 128 rows laid side by side as a [128, 128]
block Y; the device computes Y^T @ Y, whose four diagonal [32,32] blocks
are the four tiles' Gram contributions (off-diagonal blocks are
discarded).  One LDWEIGHTS per 512 samples instead of per 128, and a
128-wide moving operand, keep the PE near its streaming floor
(~56ns/quad); classes are then paced by the ~310GB/s per-core DMA
stream.  Per-class Grams accumulate across a class's quads in PSUM; each
finished accumulator is copied to SBUF by the idle Vector engine and
DMA'd out per class so only the last class's copy sits on the tail.
Host sums the diagonal blocks over cores in float64 and evaluates the
32x32 logdets there, exactly like the reference.

DMA only covers each class's real rows (tile-aligned); the padding tail
of each SBUF chunk is zeroed once by the idle Vector engine, so the
program shape depends on the label histogram (compiled per input —
inputs are deterministic, so this compiles once).
"""

import sys

sys.path.insert(0, "/opt/trn_rl_repo")

import ml_dtypes
import numpy as np

import concourse.bacc as bacc
import concourse.mybir as mybir
import concourse.tile as tile
from concourse.bass_utils import run_bass_kernel_spmd

N, D, C = 600000, 32, 10
EPS = 0.5
NCORES = 8
QC = 15                      # quads per (core, class): capacity 15*512 = 7680 rows
QROWS = 512                  # samples per quad (4 tiles of 128)
CW = QC * 128                # 1920 SBUF columns per class chunk
TOTW = C * CW                # 19200 columns of packed input per core

_cache = {}


def _build_program(tiles_per_class):
    """tiles_per_class[j] = number of real (non-padding) 128-row tiles for
    class j on every core (max over cores), <= 4*QC."""
    nc = bacc.Bacc(None)
    fp8 = mybir.dt.float8e4
    f32 = mybir.dt.float32

    z_dram = nc.dram_tensor("ZP", [128, TOTW], fp8, kind="ExternalInput")
    out_dram = nc.dram_tensor(
        "grams", [128, C * 128], f32, kind="ExternalOutput"
    )

    # chunk schedule in quads: small first chunks so compute starts early,
    # then one chunk per class; all prefetched upfront (no slot reuse).
    chunks = [(0, 0, 8), (0, 8, 7)] + [(j, 0, QC) for j in range(1, C)]

    with tile.TileContext(nc) as tc:
        with (
            tc.tile_pool(name="zin", bufs=1) as zin_pool,
            tc.tile_pool(name="outp", bufs=2) as out_pool,
            tc.tile_pool(name="psum", bufs=4, space="PSUM") as psum_pool,
        ):
            # PE p-state warmup: matmul a zeroed tile while the first
            # chunk is still streaming in.
            w_sb = out_pool.tile([128, 128], fp8, bufs=1)
            nc.gpsimd.memset(w_sb[:], 0)
            wacc = psum_pool.tile([128, 32], f32, tag="warm", bufs=1)
            for _ in range(8):
                nc.tensor.matmul(
                    wacc[:], w_sb[:], w_sb[:, 0:32], start=True, stop=True
                )

            class_tiles = {}  # class -> list of (tile, quad offset in tile)
            memsets = []
            for ci, (j, q0, nq) in enumerate(chunks):
                z_sb = zin_pool.tile(
                    [128, nq * 128], fp8, tag=f"z{ci}", bufs=1, name=f"zc{ci}"
                )
                # real data covers tiles [0, tj) of class j; this chunk holds
                # tiles [4*q0, 4*(q0+nq)).
                tj = tiles_per_class[j]
                lo_t, hi_t = 4 * q0, 4 * (q0 + nq)
                ncols = (min(tj, hi_t) - lo_t) * D  # may be <= 0
                if ncols > 0:
                    nc.sync.dma_start(
                        z_sb[:, :ncols],
                        z_dram[:, j * CW + q0 * 128 : j * CW + q0 * 128 + ncols],
                    )
                if ncols < nq * 128:
                    memsets.append(z_sb[:, max(ncols, 0) :])
                class_tiles.setdefault(j, [])
                for q in range(nq):
                    class_tiles[j].append((z_sb, q))

            for m in memsets:
                nc.vector.memset(m, 0)

            for j in range(C):
                acc = psum_pool.tile([128, 128], f32, tag="acc")
                for q in range(QC):
                    z_sb, qo = class_tiles[j][q]
                    nc.tensor.matmul(
                        acc[:],
                        z_sb[:, qo * 128 : (qo + 1) * 128],
                        z_sb[:, qo * 128 : (qo + 1) * 128],
                        start=(q == 0),
                        stop=(q + 1 == QC),
                    )
                o_j = out_pool.tile(
                    [128, 128], f32, tag=f"o{j}", bufs=1, name=f"oc{j}"
                )
                nc.vector.tensor_copy(o_j[:], acc[:])
                nc.sync.dma_start(
                    out_dram[:, j * 128 : (j + 1) * 128], o_j[:]
                )

    nc.compile()
    return nc


def kernel(Z: np.ndarray, labels: np.ndarray) -> np.ndarray:
    Z = np.asarray(Z, dtype=np.float32)
    labels = np.asarray(labels, dtype=np.int32)
    n = Z.shape[0]

    counts = np.bincount(labels, minlength=C)
    assert counts.max() <= NCORES * QC * QROWS, "class capacity exceeded"
    order = np.argsort(labels, kind="stable")
    Zs = Z[order].astype(ml_dtypes.float8_e4m3)
    starts = np.concatenate([[0], np.cumsum(counts)])

    # per-class per-core row counts; DMA covers the max tile count over cores
    # so one program serves all 8 cores.
    tiles_per_class = []
    for j in range(C):
        mx = 0
        for k in range(NCORES):
            nrows = (k + 1) * counts[j] // NCORES - k * counts[j] // NCORES
            mx = max(mx, nrows)
        tiles_per_class.append(min((mx + 127) // 128, 4 * QC))

    key = tuple(tiles_per_class)
    if _cache.get("key") != key:
        _cache["nc"] = _build_program(tiles_per_class)
        _cache["key"] = key
    nc = _cache["nc"]

    in_maps = []
    for k in range(NCORES):
        buf = np.zeros([C, QC * QROWS, D], ml_dtypes.float8_e4m3)
        for j in range(C):
            lo = starts[j] + k * counts[j] // NCORES
            hi = starts[j] + (k + 1) * counts[j] // NCORES
            buf[j, : hi - lo] = Zs[lo:hi]
        # pack: class j, quad q -> Y[p, 32*t+f] = rows[q*512 + t*128 + p, f]
        a = (
            buf.reshape(C, QC, 4, 128, D)
            .transpose(3, 0, 1, 2, 4)
            .reshape(128, TOTW)
        )
        in_maps.append({"ZP": np.ascontiguousarray(a)})

    res = run_bass_kernel_spmd(nc, in_maps, core_ids=list(range(NCORES)))
    _cache["last_results"] = res

    gj = np.zeros([C, D, D], np.float64)
    for r in res.results:
        g = r["grams"].astype(np.float64).reshape(128, C, 128)
        for j in range(C):
            for t in range(4):
                gj[j] += g[t * D : (t + 1) * D, j, t * D : (t + 1) * D]

    g_all = gj.sum(axis=0)
    tr_pi = counts.astype(np.float64)

    nf, df = float(n), float(D)
    eye = np.eye(D)
    loss_r = 0.5 * np.linalg.slogdet(eye + (df / (nf * EPS)) * g_all)[1]
    loss_rc = 0.0
    for j in range(C):
        ld = np.linalg.slogdet(eye + (df / (tr_pi[j] * EPS)) * gj[j])[1]
        loss_rc += (tr_pi[j] / (2.0 * nf)) * ld
    loss_obj = loss_r - loss_rc
    return np.asarray([-loss_obj, loss_r, loss_rc], dtype=np.float32)


# revision 37
# speedup vs baseline: 1.0268x; 1.0268x over previous
"""MCR2 loss kernel for 8 Trainium2 NeuronCores.

Host-side counting sort by class label removes all masking work from the
device: each core receives its share of every class's rows, zero-padded
to 512-row (quad) alignment and pre-packed in fp8 e4m3 (the logdet
difference loss_R - loss_Rc cancels the correlated quantization error;
measured rel err 3.1e-3, on par with bf16, for half the HBM traffic).
A quad is 4 sample tiles of 128 rows laid side by side as a [128, 128]
block Y; the device computes Y^T @ Y, whose four diagonal [32,32] blocks
are the four tiles' Gram contributions (off-diagonal blocks are
discarded).  One LDWEIGHTS per 512 samples instead of per 128, and a
128-wide moving operand, keep the PE near its streaming floor
(~56ns/quad); classes are then paced by the ~310GB/s per-core DMA
stream.  Per-class Grams accumulate across a class's quads in PSUM; each
finished accumulator is copied to SBUF by the idle Vector engine and
DMA'd out per class so only the last class's copy sits on the tail.
Host sums the diagonal blocks over cores in float64 and evaluates the
32x32 logdets there, exactly like the reference.

DMA only covers each class's real rows (tile-aligned); the padding tail
of each SBUF chunk is zeroed once by the idle Vector engine, so the
program shape depends on the label histogram (compiled per input —
inputs are deterministic, so this compiles once).
"""

import sys

sys.path.insert(0, "/opt/trn_rl_repo")

import ml_dtypes
import numpy as np

import concourse.bacc as bacc
import concourse.mybir as mybir
import concourse.tile as tile
from concourse.bass_utils import run_bass_kernel_spmd

N, D, C = 600000, 32, 10
EPS = 0.5
NCORES = 8
QC = 15                      # quads per (core, class): capacity 15*512 = 7680 rows
QROWS = 512                  # samples per quad (4 tiles of 128)
CW = QC * 128                # 1920 SBUF columns per class chunk
TOTW = C * CW                # 19200 columns of packed input per core

_cache = {}


def _build_program(tiles_per_class):
    """tiles_per_class[j] = number of real (non-padding) 128-row tiles for
    class j on every core (max over cores), <= 4*QC."""
    nc = bacc.Bacc(None)
    fp8 = mybir.dt.float8e4
    f32 = mybir.dt.float32

    z_dram = nc.dram_tensor("ZP", [128, TOTW], fp8, kind="ExternalInput")
    out_dram = nc.dram_tensor(
        "grams", [128, C * 128], f32, kind="ExternalOutput"
    )

    # chunk schedule in quads: small first chunks so compute starts early,
    # then one chunk per class; all prefetched upfront (no slot reuse).
    # chunk0 is split in half across Sync and GpSimd: the two DMA rings
    # pay their cold-start latency in parallel, so class 0's data lands
    # ~0.5us earlier.  Everything else streams in order on Sync's ring.
    chunks = [(0, 0, 4, "s"), (0, 4, 4, "g"), (0, 8, 7, "s")] + [
        (j, 0, QC, "s") for j in range(1, C)
    ]

    with tile.TileContext(nc) as tc:
        with (
            tc.tile_pool(name="zin", bufs=1) as zin_pool,
            tc.tile_pool(name="outp", bufs=2) as out_pool,
            tc.tile_pool(name="psum", bufs=4, space="PSUM") as psum_pool,
        ):
            # PE p-state warmup: matmul a zeroed tile while the first
            # chunk is still streaming in.
            w_sb = out_pool.tile([128, 128], fp8, bufs=1)
            nc.gpsimd.memset(w_sb[:], 0)
            wacc = psum_pool.tile([128, 32], f32, tag="warm", bufs=1)
            for _ in range(8):
                nc.tensor.matmul(
                    wacc[:], w_sb[:], w_sb[:, 0:32], start=True, stop=True
                )

            class_tiles = {}  # class -> list of (tile, quad offset in tile)
            memsets = []
            for ci, (j, q0, nq, eng) in enumerate(chunks):
                z_sb = zin_pool.tile(
                    [128, nq * 128], fp8, tag=f"z{ci}", bufs=1, name=f"zc{ci}"
                )
                # real data covers tiles [0, tj) of class j; this chunk holds
                # tiles [4*q0, 4*(q0+nq)).
                tj = tiles_per_class[j]
                lo_t, hi_t = 4 * q0, 4 * (q0 + nq)
                ncols = (min(tj, hi_t) - lo_t) * D  # may be <= 0
                if ncols > 0:
                    (nc.sync if eng == "s" else nc.gpsimd).dma_start(
                        z_sb[:, :ncols],
                        z_dram[:, j * CW + q0 * 128 : j * CW + q0 * 128 + ncols],
                    )
                if ncols < nq * 128:
                    memsets.append(z_sb[:, max(ncols, 0) :])
                class_tiles.setdefault(j, [])
                for q in range(nq):
                    class_tiles[j].append((z_sb, q))

            for m in memsets:
                nc.vector.memset(m, 0)

            for j in range(C):
                acc = psum_pool.tile([128, 128], f32, tag="acc")
                for q in range(QC):
                    z_sb, qo = class_tiles[j][q]
                    nc.tensor.matmul(
                        acc[:],
                        z_sb[:, qo * 128 : (qo + 1) * 128],
                        z_sb[:, qo * 128 : (qo + 1) * 128],
                        start=(q == 0),
                        stop=(q + 1 == QC),
                    )
                o_j = out_pool.tile(
                    [128, 128], f32, tag=f"o{j}", bufs=1, name=f"oc{j}"
                )
                nc.vector.tensor_copy(o_j[:], acc[:])
                nc.sync.dma_start(
                    out_dram[:, j * 128 : (j + 1) * 128], o_j[:]
                )

    nc.compile()
    return nc


def kernel(Z: np.ndarray, labels: np.ndarray) -> np.ndarray:
    Z = np.asarray(Z, dtype=np.float32)
    labels = np.asarray(labels, dtype=np.int32)
    n = Z.shape[0]

    counts = np.bincount(labels, minlength=C)
    assert counts.max() <= NCORES * QC * QROWS, "class capacity exceeded"
    order = np.argsort(labels, kind="stable")
    Zs = Z[order].astype(ml_dtypes.float8_e4m3)
    starts = np.concatenate([[0], np.cumsum(counts)])

    # per-class per-core row counts; DMA covers the max tile count over cores
    # so one program serves all 8 cores.
    tiles_per_class = []
    for j in range(C):
        mx = 0
        for k in range(NCORES):
            nrows = (k + 1) * counts[j] // NCORES - k * counts[j] // NCORES
            mx = max(mx, nrows)
        tiles_per_class.append(min((mx + 127) // 128, 4 * QC))

    key = tuple(tiles_per_class)
    if _cache.get("key") != key:
        _cache["nc"] = _build_program(tiles_per_class)
        _cache["key"] = key
    nc = _cache["nc"]

    in_maps = []
    for k in range(NCORES):
        buf = np.zeros([C, QC * QROWS, D], ml_dtypes.float8_e4m3)
        for j in range(C):
            lo = starts[j] + k * counts[j] // NCORES
            hi = starts[j] + (k + 1) * counts[j] // NCORES
            buf[j, : hi - lo] = Zs[lo:hi]
        # pack: class j, quad q -> Y[p, 32*t+f] = rows[q*512 + t*128 + p, f]
        a = (
            buf.reshape(C, QC, 4, 128, D)
            .transpose(3, 0, 1, 2, 4)
            .reshape(128, TOTW)
        )
        in_maps.append({"ZP": np.ascontiguousarray(a)})

    res = run_bass_kernel_spmd(nc, in_maps, core_ids=list(range(NCORES)))
    _cache["last_results"] = res

    gj = np.zeros([C, D, D], np.float64)
    for r in res.results:
        g = r["grams"].astype(np.float64).reshape(128, C, 128)
        for j in range(C):
            for t in range(4):
                gj[j] += g[t * D : (t + 1) * D, j, t * D : (t + 1) * D]

    g_all = gj.sum(axis=0)
    tr_pi = counts.astype(np.float64)

    nf, df = float(n), float(D)
    eye = np.eye(D)
    loss_r = 0.5 * np.linalg.slogdet(eye + (df / (nf * EPS)) * g_all)[1]
    loss_rc = 0.0
    for j in range(C):
        ld = np.linalg.slogdet(eye + (df / (tr_pi[j] * EPS)) * gj[j])[1]
        loss_rc += (tr_pi[j] / (2.0 * nf)) * ld
    loss_obj = loss_r - loss_rc
    return np.asarray([-loss_obj, loss_r, loss_rc], dtype=np.float32)


# revision 38
# speedup vs baseline: 1.0271x; 1.0002x over previous
"""MCR2 loss kernel for 8 Trainium2 NeuronCores.

Host-side counting sort by class label removes all masking work from the
device: each core receives its share of every class's rows, zero-padded
to 512-row (quad) alignment and pre-packed in fp8 e4m3 (the logdet
difference loss_R - loss_Rc cancels the correlated quantization error;
measured rel err 3.1e-3, on par with bf16, for half the HBM traffic).
A quad is 4 sample tiles of 128 rows laid side by side as a [128, 128]
block Y; the device computes Y^T @ Y, whose four diagonal [32,32] blocks
are the four tiles' Gram contributions (off-diagonal blocks are
discarded).  One LDWEIGHTS per 512 samples instead of per 128, and a
128-wide moving operand, keep the PE near its streaming floor
(~56ns/quad); classes are then paced by the ~310GB/s per-core DMA
stream.  Per-class Grams accumulate across a class's quads in PSUM; each
finished accumulator is copied to SBUF by the idle Vector engine and
DMA'd out per class so only the last class's copy sits on the tail.
Host sums the diagonal blocks over cores in float64 and evaluates the
32x32 logdets there, exactly like the reference.

DMA only covers each class's real rows (tile-aligned); the padding tail
of each SBUF chunk is zeroed once by the idle Vector engine, so the
program shape depends on the label histogram (compiled per input —
inputs are deterministic, so this compiles once).
"""

import sys

sys.path.insert(0, "/opt/trn_rl_repo")

import ml_dtypes
import numpy as np

import concourse.bacc as bacc
import concourse.mybir as mybir
import concourse.tile as tile
from concourse.bass_utils import run_bass_kernel_spmd

N, D, C = 600000, 32, 10
EPS = 0.5
NCORES = 8
QC = 15                      # quads per (core, class): capacity 15*512 = 7680 rows
QROWS = 512                  # samples per quad (4 tiles of 128)
CW = QC * 128                # 1920 SBUF columns per class chunk
TOTW = C * CW                # 19200 columns of packed input per core

_cache = {}


def _build_program(tiles_per_class):
    """tiles_per_class[j] = number of real (non-padding) 128-row tiles for
    class j on every core (max over cores), <= 4*QC."""
    nc = bacc.Bacc(None)
    fp8 = mybir.dt.float8e4
    f32 = mybir.dt.float32

    z_dram = nc.dram_tensor("ZP", [128, TOTW], fp8, kind="ExternalInput")
    out_dram = nc.dram_tensor(
        "grams", [128, C * 128], f32, kind="ExternalOutput"
    )

    # chunk schedule in quads: small first chunks so compute starts early,
    # then one chunk per class; all prefetched upfront (no slot reuse).
    # chunk0 is split in half across Sync and GpSimd: the two DMA rings
    # pay their cold-start latency in parallel, so class 0's data lands
    # ~0.5us earlier.  Everything else streams in order on Sync's ring.
    chunks = [(0, 0, 4, "s"), (0, 4, 4, "g"), (0, 8, 7, "s")] + [
        (j, 0, QC, "s") for j in range(1, C)
    ]

    with tile.TileContext(nc) as tc:
        with (
            tc.tile_pool(name="zin", bufs=1) as zin_pool,
            tc.tile_pool(name="outp", bufs=2) as out_pool,
            tc.tile_pool(name="psum", bufs=4, space="PSUM") as psum_pool,
        ):
            # PE p-state warmup: matmul a zeroed tile while the first
            # chunk is still streaming in.
            w_sb = out_pool.tile([128, 128], fp8, bufs=1)
            nc.vector.memset(w_sb[:], 0)
            wacc = psum_pool.tile([128, 32], f32, tag="warm", bufs=1)
            for _ in range(8):
                nc.tensor.matmul(
                    wacc[:], w_sb[:], w_sb[:, 0:32], start=True, stop=True
                )

            class_tiles = {}  # class -> list of (tile, quad offset in tile)
            memsets = []
            for ci, (j, q0, nq, eng) in enumerate(chunks):
                z_sb = zin_pool.tile(
                    [128, nq * 128], fp8, tag=f"z{ci}", bufs=1, name=f"zc{ci}"
                )
                # real data covers tiles [0, tj) of class j; this chunk holds
                # tiles [4*q0, 4*(q0+nq)).
                tj = tiles_per_class[j]
                lo_t, hi_t = 4 * q0, 4 * (q0 + nq)
                ncols = (min(tj, hi_t) - lo_t) * D  # may be <= 0
                if ncols > 0:
                    (nc.sync if eng == "s" else nc.gpsimd).dma_start(
                        z_sb[:, :ncols],
                        z_dram[:, j * CW + q0 * 128 : j * CW + q0 * 128 + ncols],
                    )
                if ncols < nq * 128:
                    memsets.append(z_sb[:, max(ncols, 0) :])
                class_tiles.setdefault(j, [])
                for q in range(nq):
                    class_tiles[j].append((z_sb, q))

            for m in memsets:
                nc.vector.memset(m, 0)

            for j in range(C):
                acc = psum_pool.tile([128, 128], f32, tag="acc")
                for q in range(QC):
                    z_sb, qo = class_tiles[j][q]
                    nc.tensor.matmul(
                        acc[:],
                        z_sb[:, qo * 128 : (qo + 1) * 128],
                        z_sb[:, qo * 128 : (qo + 1) * 128],
                        start=(q == 0),
                        stop=(q + 1 == QC),
                    )
                o_j = out_pool.tile(
                    [128, 128], f32, tag=f"o{j}", bufs=1, name=f"oc{j}"
                )
                nc.vector.tensor_copy(o_j[:], acc[:])
                nc.sync.dma_start(
                    out_dram[:, j * 128 : (j + 1) * 128], o_j[:]
                )

    nc.compile()
    return nc


def kernel(Z: np.ndarray, labels: np.ndarray) -> np.ndarray:
    Z = np.asarray(Z, dtype=np.float32)
    labels = np.asarray(labels, dtype=np.int32)
    n = Z.shape[0]

    counts = np.bincount(labels, minlength=C)
    assert counts.max() <= NCORES * QC * QROWS, "class capacity exceeded"
    order = np.argsort(labels, kind="stable")
    Zs = Z[order].astype(ml_dtypes.float8_e4m3)
    starts = np.concatenate([[0], np.cumsum(counts)])

    # per-class per-core row counts; DMA covers the max tile count over cores
    # so one program serves all 8 cores.
    tiles_per_class = []
    for j in range(C):
        mx = 0
        for k in range(NCORES):
            nrows = (k + 1) * counts[j] // NCORES - k * counts[j] // NCORES
            mx = max(mx, nrows)
        tiles_per_class.append(min((mx + 127) // 128, 4 * QC))

    key = tuple(tiles_per_class)
    if _cache.get("key") != key:
        _cache["nc"] = _build_program(tiles_per_class)
        _cache["key"] = key
    nc = _cache["nc"]

    in_maps = []
    for k in range(NCORES):
        buf = np.zeros([C, QC * QROWS, D], ml_dtypes.float8_e4m3)
        for j in range(C):
            lo = starts[j] + k * counts[j] // NCORES
            hi = starts[j] + (k + 1) * counts[j] // NCORES
            buf[j, : hi - lo] = Zs[lo:hi]
        # pack: class j, quad q -> Y[p, 32*t+f] = rows[q*512 + t*128 + p, f]
        a = (
            buf.reshape(C, QC, 4, 128, D)
            .transpose(3, 0, 1, 2, 4)
            .reshape(128, TOTW)
        )
        in_maps.append({"ZP": np.ascontiguousarray(a)})

    res = run_bass_kernel_spmd(nc, in_maps, core_ids=list(range(NCORES)))
    _cache["last_results"] = res

    gj = np.zeros([C, D, D], np.float64)
    for r in res.results:
        g = r["grams"].astype(np.float64).reshape(128, C, 128)
        for j in range(C):
            for t in range(4):
                gj[j] += g[t * D : (t + 1) * D, j, t * D : (t + 1) * D]

    g_all = gj.sum(axis=0)
    tr_pi = counts.astype(np.float64)

    nf, df = float(n), float(D)
    eye = np.eye(D)
    loss_r = 0.5 * np.linalg.slogdet(eye + (df / (nf * EPS)) * g_all)[1]
    loss_rc = 0.0
    for j in range(C):
        ld = np.linalg.slogdet(eye + (df / (tr_pi[j] * EPS)) * gj[j])[1]
        loss_rc += (tr_pi[j] / (2.0 * nf)) * ld
    loss_obj = loss_r - loss_rc
    return np.asarray([-loss_obj, loss_r, loss_rc], dtype=np.float32)


# revision 39
# speedup vs baseline: 1.0551x; 1.0273x over previous
"""MCR2 loss kernel for 8 Trainium2 NeuronCores.

Host-side counting sort by class label removes all masking work from the
device: each core receives its share of every class's rows, zero-padded
to 512-row (quad) alignment and pre-packed in fp8 e4m3 (the logdet
difference loss_R - loss_Rc cancels the correlated quantization error;
measured rel err 3.1e-3, on par with bf16, for half the HBM traffic).
A quad is 4 sample tiles of 128 rows laid side by side as a [128, 128]
block Y; the device computes Y^T @ Y, whose four diagonal [32,32] blocks
are the four tiles' Gram contributions (off-diagonal blocks are
discarded).  One LDWEIGHTS per 512 samples instead of per 128, and a
128-wide moving operand, keep the PE near its streaming floor
(~56ns/quad); classes are then paced by the ~310GB/s per-core DMA
stream.  Per-class Grams accumulate across a class's quads in PSUM; each
finished accumulator is copied to SBUF by the idle Vector engine and
DMA'd out per class so only the last class's copy sits on the tail.
Host sums the diagonal blocks over cores in float64 and evaluates the
32x32 logdets there, exactly like the reference.

DMA only covers each class's real rows (tile-aligned); the padding tail
of each SBUF chunk is zeroed once by the idle Vector engine, so the
program shape depends on the label histogram (compiled per input —
inputs are deterministic, so this compiles once).
"""

import sys

sys.path.insert(0, "/opt/trn_rl_repo")

import ml_dtypes
import numpy as np

import concourse.bacc as bacc
import concourse.mybir as mybir
import concourse.tile as tile
from concourse.bass_utils import run_bass_kernel_spmd

N, D, C = 600000, 32, 10
EPS = 0.5
NCORES = 8
QC = 15                      # quads per (core, class): capacity 15*512 = 7680 rows
QROWS = 512                  # samples per quad (4 tiles of 128)
CW = QC * 128                # 1920 SBUF columns per class chunk
TOTW = C * CW                # 19200 columns of packed input per core

_cache = {}


def _build_program(tiles_per_class):
    """tiles_per_class[j] = number of real (non-padding) 128-row tiles for
    class j on every core (max over cores), <= 4*QC."""
    nc = bacc.Bacc(None)
    fp8 = mybir.dt.float8e4
    f32 = mybir.dt.float32

    z_dram = nc.dram_tensor("ZP", [128, TOTW], fp8, kind="ExternalInput")
    out_dram = nc.dram_tensor(
        "grams", [128, C * 128], f32, kind="ExternalOutput"
    )

    # chunk schedule in quads: small first chunks so compute starts early,
    # then one chunk per class; all prefetched upfront (no slot reuse).
    chunks = [(0, 0, 8, "s"), (0, 8, 7, "s")] + [
        (j, 0, QC, "s") for j in range(1, C)
    ]

    with tile.TileContext(nc) as tc:
        with (
            tc.tile_pool(name="zin", bufs=1) as zin_pool,
            tc.tile_pool(name="outp", bufs=2) as out_pool,
            tc.tile_pool(name="psum", bufs=4, space="PSUM") as psum_pool,
        ):
            # PE p-state warmup: matmul a zeroed tile while the first
            # chunk is still streaming in.
            w_sb = out_pool.tile([128, 128], fp8, bufs=1)
            nc.gpsimd.memset(w_sb[:], 0)
            wacc = psum_pool.tile([128, 32], f32, tag="warm", bufs=1)
            for _ in range(8):
                nc.tensor.matmul(
                    wacc[:], w_sb[:], w_sb[:, 0:32], start=True, stop=True
                )

            class_tiles = {}  # class -> list of (tile, quad offset in tile)
            memsets = []
            for ci, (j, q0, nq, eng) in enumerate(chunks):
                z_sb = zin_pool.tile(
                    [128, nq * 128], fp8, tag=f"z{ci}", bufs=1, name=f"zc{ci}"
                )
                # real data covers tiles [0, tj) of class j; this chunk holds
                # tiles [4*q0, 4*(q0+nq)).
                tj = tiles_per_class[j]
                lo_t, hi_t = 4 * q0, 4 * (q0 + nq)
                ncols = (min(tj, hi_t) - lo_t) * D  # may be <= 0
                if ncols > 0:
                    (nc.sync if eng == "s" else nc.gpsimd).dma_start(
                        z_sb[:, :ncols],
                        z_dram[:, j * CW + q0 * 128 : j * CW + q0 * 128 + ncols],
                    )
                if ncols < nq * 128:
                    memsets.append(z_sb[:, max(ncols, 0) :])
                class_tiles.setdefault(j, [])
                for q in range(nq):
                    class_tiles[j].append((z_sb, q))

            for m in memsets:
                nc.vector.memset(m, 0)

            for j in range(C):
                acc = psum_pool.tile([128, 128], f32, tag="acc")
                for q in range(QC):
                    z_sb, qo = class_tiles[j][q]
                    nc.tensor.matmul(
                        acc[:],
                        z_sb[:, qo * 128 : (qo + 1) * 128],
                        z_sb[:, qo * 128 : (qo + 1) * 128],
                        start=(q == 0),
                        stop=(q + 1 == QC),
                    )
                o_j = out_pool.tile(
                    [128, 128], f32, tag=f"o{j}", bufs=1, name=f"oc{j}"
                )
                nc.vector.tensor_copy(o_j[:], acc[:])
                nc.sync.dma_start(
                    out_dram[:, j * 128 : (j + 1) * 128], o_j[:]
                )

    nc.compile()
    return nc


def kernel(Z: np.ndarray, labels: np.ndarray) -> np.ndarray:
    Z = np.asarray(Z, dtype=np.float32)
    labels = np.asarray(labels, dtype=np.int32)
    n = Z.shape[0]

    counts = np.bincount(labels, minlength=C)
    assert counts.max() <= NCORES * QC * QROWS, "class capacity exceeded"
    order = np.argsort(labels, kind="stable")
    Zs = Z[order].astype(ml_dtypes.float8_e4m3)
    starts = np.concatenate([[0], np.cumsum(counts)])

    # per-class per-core row counts; DMA covers the max tile count over cores
    # so one program serves all 8 cores.
    tiles_per_class = []
    for j in range(C):
        mx = 0
        for k in range(NCORES):
            nrows = (k + 1) * counts[j] // NCORES - k * counts[j] // NCORES
            mx = max(mx, nrows)
        tiles_per_class.append(min((mx + 127) // 128, 4 * QC))

    key = tuple(tiles_per_class)
    if _cache.get("key") != key:
        _cache["nc"] = _build_program(tiles_per_class)
        _cache["key"] = key
    nc = _cache["nc"]

    in_maps = []
    for k in range(NCORES):
        buf = np.zeros([C, QC * QROWS, D], ml_dtypes.float8_e4m3)
        for j in range(C):
            lo = starts[j] + k * counts[j] // NCORES
            hi = starts[j] + (k + 1) * counts[j] // NCORES
            buf[j, : hi - lo] = Zs[lo:hi]
        # pack: class j, quad q -> Y[p, 32*t+f] = rows[q*512 + t*128 + p, f]
        a = (
            buf.reshape(C, QC, 4, 128, D)
            .transpose(3, 0, 1, 2, 4)
            .reshape(128, TOTW)
        )
        in_maps.append({"ZP": np.ascontiguousarray(a)})

    res = run_bass_kernel_spmd(nc, in_maps, core_ids=list(range(NCORES)))
    _cache["last_results"] = res

    gj = np.zeros([C, D, D], np.float64)
    for r in res.results:
        g = r["grams"].astype(np.float64).reshape(128, C, 128)
        for j in range(C):
            for t in range(4):
                gj[j] += g[t * D : (t + 1) * D, j, t * D : (t + 1) * D]

    g_all = gj.sum(axis=0)
    tr_pi = counts.astype(np.float64)

    nf, df = float(n), float(D)
    eye = np.eye(D)
    loss_r = 0.5 * np.linalg.slogdet(eye + (df / (nf * EPS)) * g_all)[1]
    loss_rc = 0.0
    for j in range(C):
        ld = np.linalg.slogdet(eye + (df / (tr_pi[j] * EPS)) * gj[j])[1]
        loss_rc += (tr_pi[j] / (2.0 * nf)) * ld
    loss_obj = loss_r - loss_rc
    return np.asarray([-loss_obj, loss_r, loss_rc], dtype=np.float32)
